# revision 52
# baseline (speedup 1.0000x reference)
"""MetaCA Trainium2 kernel: 8-core data-parallel (one batch row per core).

fp8 DoubleRow version. Cells kept resident in SBUF in two forms:
  - f16 working state [128, T+2] (blend input/output, LN input)
  - fp8 dual-image [128, 2, LDUP]: plane0[c] = x[c-1], plane1[c] = x[c-2]
    so a single 3D AP [128, 2, 512] feeds DoubleRow matmuls with the
    (center, left) K=256 contraction; plane stride LDUP = 4112 B (16-aligned).

Per evolve iteration, per 1024-token macro-tile, per rule:
  GEMM1: 4x DR MM (K=256 center+left) + 4x plain fp8 MM (K=128 right)
         into PSUM [128, 2048]; weights pre-scaled x16 for fp8 range
  GELU (ACT): one [128,2048] op, scale=1/16 free affine, writes fp8 h2
  GEMM2: per rule 2x DR MM (K=256 exact)   [pairs of rules share PSUM tile]
  TANH (ACT): one [128,2048] op per rule pair, scale=1/16, writes f16
  R-sum (DVE, f16 2x): acc += (1-alpha)*w_r * t_r
  blend (DVE): nxt16 = alpha*cur16 + acc;  GPSIMD copies the two fp8 images
GEMM2 pairs are delayed ~2 rules behind GEMM1 so the PE never waits on ACT.
Selector MLPs run on host in float64. Final LayerNorm: f16 PE transposes,
with per-token mean sums computed on the PE (ones-vector N=1 matmuls reusing
the transpose stationaries); Square (ACT) + one reduce (DVE) for variance;
applies split ACT/DVE; output staged partition-major in f16 (2KB-contiguous
DMA chunks, host un-permutes and casts to f32).
Head DMA triggers ride the sync+gpsimd queues so the ACT queue stays clear.
"""

import numpy as np
from contextlib import ExitStack

import concourse.bass as bass
import concourse.bacc as bacc
import concourse.mybir as mybir
from concourse.tile import TileContext
from concourse.bass_utils import run_bass_kernel_spmd
from concourse.masks import make_identity

B, T, D, R = 8, 4096, 128, 8
H2 = 2 * D            # 256 hidden per rule
M1 = R * H2           # 2048 GEMM1 output features
LN_EPS = 1e-5
TT = 1024             # macro token tile
NMT = T // TT         # 4 macro tiles
LDUP = 4112           # fp8 image plane stride (>= T+2, multiple of 16)
WS = 16.0             # fp8 weight pre-scale
# tanh(y) ~ y*(PA + PB*y^2 + PC*y^4) on |y|<=0.9 (maxerr 4.8e-4; measured
# |y|max over the run is 0.52); scaled for the z = WS*y PSUM values
PA = 0.998617111019936 / WS
PB = -0.3170009922437158 / (WS * WS * WS)
PC = 0.0831769179849738 / (WS ** 5)
F32 = mybir.dt.float32
F16 = mybir.dt.float16
F8 = mybir.dt.float8e4
AF = mybir.ActivationFunctionType
OP = mybir.AluOpType
DRM = mybir.MatmulPerfMode.DoubleRow


def _gelu64(x):
    try:
        from scipy.special import erf
    except ImportError:
        import math
        erf = np.vectorize(math.erf, otypes=[np.float64])
    return 0.5 * x * (1.0 + erf(x / np.sqrt(2.0)))


def _softmax64(v):
    e = np.exp(v - v.max())
    return e / e.sum()


def _selectors(inputs):
    f = lambda k: np.asarray(inputs[k], np.float64)
    c = f("c_state")

    def mlp(p):
        return _gelu64(c @ f(p + "_W1") + f(p + "_b1")) @ f(p + "_W2") + f(p + "_b2")

    rw = _softmax64(mlp("rsel"))
    sw = _softmax64(mlp("ssel"))
    n_soft = float((sw * np.arange(2.0, 9.0)).sum())
    n_evolve = max(2, min(8, int(n_soft + 0.5)))
    alpha = float(0.1 + 0.8 / (1.0 + np.exp(-mlp("asel")[0])))
    return [float(w) for w in rw], alpha, n_evolve


def build_nc(n_evolve, alpha, rule_w, apply_gb=False):
    nc = bacc.Bacc("TRN2", target_bir_lowering=False, debug=False)
    x16_d = nc.declare_dram_parameter("x16", [128, T], F16, isOutput=False)
    ximg_d = nc.declare_dram_parameter("ximg", [128, 2, LDUP], F8, isOutput=False)
    w1_d = nc.declare_dram_parameter("w1", [128, 3, M1], F8, isOutput=False)
    w2_d = nc.declare_dram_parameter("w2", [128, 2, R * D], F8, isOutput=False)
    if apply_gb:
        gb_d = nc.declare_dram_parameter("gb", [2, 128, D], F32, isOutput=False)
    # partition-major staging layout: y[p, jj, d] holds token jj*128+p.
    # 2KB-contiguous per partition per quarter-DMA (8x fewer DMA packets
    # than token-major); the host un-permutes, which is free.
    y_d = nc.declare_dram_parameter("y", [128, T // 128, D], F16, isOutput=True)

    wp = [w * (1.0 - alpha) for w in rule_w]   # fold (1-alpha) into rule weights

    with ExitStack() as ctx:
        tc = ctx.enter_context(TileContext(nc))
        cpool = ctx.enter_context(tc.tile_pool(name="const", bufs=1))
        cellp = ctx.enter_context(tc.tile_pool(name="cells", bufs=1))
        hpool = ctx.enter_context(tc.tile_pool(name="hid", bufs=4))
        tpool = ctx.enter_context(tc.tile_pool(name="tanh", bufs=3))
        apool = ctx.enter_context(tc.tile_pool(name="accp", bufs=2))
        lnp = ctx.enter_context(tc.tile_pool(name="ln", bufs=3))

        w1_sb = cpool.tile([128, 3, M1], F8, tag="w1")
        w2_sb = cpool.tile([128, 2, R * D], F8, tag="w2")
        if apply_gb:
            gb_sb = cpool.tile([128, 2 * D], F32, tag="gb")
            for k in range(2):
                nc.sync.dma_start(gb_sb[:, k * D:(k + 1) * D], gb_d[k])
        ident = cpool.tile([128, 128], F32, tag="ident")
        make_identity(nc, ident[:])
        ident16 = cpool.tile([128, 128], F16, tag="ident16")
        nc.vector.tensor_copy(ident16[:], ident[:])
        ones16 = cpool.tile([128, 1], F16, tag="ones16")
        nc.vector.memset(ones16[:], 1.0)
        # preload the gelu table set while the weight DMAs are in flight
        junk = cpool.tile([128, 1], F32, tag="junk")
        nc.scalar.activation(junk[:], ident[:, 0:1], AF.Gelu)

        bufA = cellp.tile([128, T + 2], F16, tag="bufA")
        bufB = cellp.tile([128, T + 2], F16, tag="bufB")
        bufA8 = cellp.tile([128, 2, LDUP], F8, tag="bufA8")
        bufB8 = cellp.tile([128, 2, LDUP], F8, tag="bufB8")

        def mm(out, lhsT, rhs, start, stop, pm=None):
            nc.tensor.matmul(out, lhsT, rhs, start=start, stop=stop, perf_mode=pm)

        # state comes pre-transposed (f16) and pre-quantized (fp8 dual image,
        # halos included) from the host. Parallelize across both HWDGE queues;
        # the fp8 images go first (GEMM1 needs them, the f16 only at blend).
        # DMA order = consumption order: rule-0-3 weights + first token chunk
        # land first so GEMM1 starts ~3 us after the queues open
        # triggers on sync + vector queues only: the scalar (ACT) queue must
        # stay clear so the first gelus issue as soon as data lands
        nc.sync.dma_start(w1_sb[:, :, 0:256], w1_d[:, :, 0:256])
        nc.gpsimd.dma_start(bufA8[:, 1:2, 0:544], ximg_d[:, 1:2, 0:544])
        nc.sync.dma_start(bufA8[:, 0:1, 0:544], ximg_d[:, 0:1, 0:544])
        nc.gpsimd.dma_start(bufA8[:, 1:2, 544:1056], ximg_d[:, 1:2, 544:1056])
        nc.sync.dma_start(w1_sb[:, :, 256:1024], w1_d[:, :, 256:1024])
        nc.gpsimd.dma_start(bufA8[:, 0:1, 544:1056], ximg_d[:, 0:1, 544:1056])
        nc.gpsimd.dma_start(w2_sb[:], w2_d[:])
        nc.sync.dma_start(w1_sb[:, :, 1024:2048], w1_d[:, :, 1024:2048])
        cuts = (1056, 2080, 3104, LDUP)
        for c0, c1 in zip(cuts[:-1], cuts[1:]):
            nc.sync.dma_start(bufA8[:, 0:1, c0:c1], ximg_d[:, 0:1, c0:c1])
            nc.gpsimd.dma_start(bufA8[:, 1:2, c0:c1], ximg_d[:, 1:2, c0:c1])
        nc.sync.dma_start(bufA[:, 1:T // 2 + 1], x16_d[:, 0:T // 2])
        nc.gpsimd.dma_start(bufA[:, T // 2 + 1:T + 1], x16_d[:, T // 2:T])

        with tc.tile_pool(name="psum", bufs=2, space="PSUM") as ppool:

            # ---- evolve iterations ----
            cur, nxt = bufA, bufB
            cur8, nxt8 = bufA8, bufB8

            def g2_pair(pair_mt, a, h2a, h2b, acc, cur_, nxt_, nxt8_,
                        split, fixups, uid, images=True):
                """GEMM2 + tanh + R-sum for rules (a, a+1); a == 6 also blends.
                split: process token columns [512:1024] first in half-size ops
                so the wrap-halo fixup (needs only token T-1) lands early."""
                t0p = pair_mt * TT
                c0 = 1 + t0p
                ps2 = ppool.tile([128, 2048], F32, tag="mm", name=f"ps2_{uid}")
                norder = (1, 0) if split else (0, 1)
                for n in norder:
                    for idx, hx in enumerate((h2a, h2b)):
                        rr = a + idx
                        outap = ps2[:, idx * TT + n * 512: idx * TT + (n + 1) * 512]
                        mm(outap, w2_sb[:, :, rr * 128:(rr + 1) * 128],
                           hx[:, :, n * 512:(n + 1) * 512], True, True, DRM)
                tt3 = tpool.tile([128, 2, TT], F16, tag="t2", name=f"t2_{uid}")
                ps3 = ps2[:].rearrange("p (two f) -> p two f", two=2)
                halves = ((768, 1024), (0, 768)) if split else ((0, TT),)
                from contextlib import nullcontext
                for (lo, hi) in halves:
                    # the short tail chunk feeds the wrap-halo fixup: keep its
                    # whole chain at high scheduler priority
                    prio = tc.high_priority() if (split and hi == TT) \
                        else nullcontext()
                    with prio:
                        nc.scalar.activation(tt3[:, :, lo:hi], ps3[:, :, lo:hi],
                                             AF.Tanh, scale=1.0 / WS)
                        if a == 0:
                            nc.vector.tensor_scalar_mul(acc[:, lo:hi],
                                                        tt3[:, 0, lo:hi], wp[0])
                        else:
                            nc.vector.scalar_tensor_tensor(
                                acc[:, lo:hi], tt3[:, 0, lo:hi], wp[a],
                                acc[:, lo:hi], OP.mult, OP.add)
                        nc.vector.scalar_tensor_tensor(
                            acc[:, lo:hi], tt3[:, 1, lo:hi], wp[a + 1],
                            acc[:, lo:hi], OP.mult, OP.add)
                        if a == 6:
                            nc.vector.scalar_tensor_tensor(
                                nxt_[:, c0 + lo:c0 + hi], cur_[:, c0 + lo:c0 + hi],
                                alpha, acc[:, lo:hi], OP.mult, OP.add)
                            if fixups and hi == TT:
                                # plane1[1] = x[T-1]: the latency-critical halo
                                nc.vector.tensor_copy(nxt8_[:, 1:2, 1:2],
                                                      nxt_[:, T:T + 1].unsqueeze(1))
                if a == 6:
                    if fixups:
                        nc.vector.tensor_copy(nxt8_[:, 0:1, T + 1:T + 2],
                                              nxt_[:, 1:2].unsqueeze(1))
                    if images:
                        nc.gpsimd.tensor_copy(nxt8_[:, 0:1, c0:c0 + TT],
                                              nxt_[:, c0:c0 + TT].unsqueeze(1))
                        nc.gpsimd.tensor_copy(nxt8_[:, 1:2, c0 + 1:c0 + TT + 1],
                                              nxt_[:, c0:c0 + TT].unsqueeze(1))

            pending = None      # deferred pair-67 closure (crosses mt/iteration)
            for it in range(n_evolve):
                h2s = [None] * R
                accs = [None] * NMT
                for mt in range(NMT):
                    t0 = mt * TT
                    accs[mt] = apool.tile([128, TT], F16, tag="acc",
                                          name=f"acc{it}_{mt}")
                    for r in range(R):
                        boundary = (mt == 0 and r == 0 and it > 0
                                    and pending is not None)
                        # GEMM1 rule r
                        ps = ppool.tile([128, 2048], F32, tag="mm",
                                        name=f"ps{it}_{mt}_{r}")
                        if boundary:
                            # n=1 chunks + n=0 plain need only old state; the
                            # deferred pair-67 chain runs before the n=0 DR MMs
                            # (which read the fresh wrap halo). The gelu is
                            # split so ACT gets the n=1 half immediately.
                            hh = hpool.tile([128, 2, TT], F8, tag="h2",
                                            name=f"h2_{it}_{mt}_{r}")
                            ps3g = ps[:].rearrange("p (hm f) -> p hm f", hm=2)
                            for hm in range(2):
                                m = 2 * r + hm
                                c0 = 1 + t0 + 512
                                outap = ps[:, hm * TT + 512: hm * TT + TT]
                                mm(outap, w1_sb[:, 0:2, m * 128:(m + 1) * 128],
                                   cur8[:, :, c0:c0 + 512], True, False, DRM)
                                mm(outap, w1_sb[:, 2:3, m * 128:(m + 1) * 128],
                                   cur8[:, 0:1, c0 + 1:c0 + 513], False, True)
                            nc.scalar.activation(hh[:, :, 512:1024],
                                                 ps3g[:, :, 512:1024],
                                                 AF.Gelu, scale=1.0 / WS)
                            for hm in range(2):
                                m = 2 * r + hm
                                c0 = 1 + t0
                                mm(ps[:, hm * TT: hm * TT + 512],
                                   w1_sb[:, 2:3, m * 128:(m + 1) * 128],
                                   cur8[:, 0:1, c0 + 1:c0 + 513], True, False)
                            pending()
                            pending = None
                            for hm in range(2):
                                m = 2 * r + hm
                                c0 = 1 + t0
                                mm(ps[:, hm * TT: hm * TT + 512],
                                   w1_sb[:, 0:2, m * 128:(m + 1) * 128],
                                   cur8[:, :, c0:c0 + 512], False, True, DRM)
                            nc.scalar.activation(hh[:, :, 0:512],
                                                 ps3g[:, :, 0:512],
                                                 AF.Gelu, scale=1.0 / WS)
                            h2s[r] = hh
                        elif mt == 0 and r == 1 and it > 0:
                            # rule 1 of the boundary mt: n=1 chunks first so
                            # only its last MMs wait on the fresh wrap halo
                            for hm in range(2):
                                m = 2 * r + hm
                                c0 = 1 + t0 + 512
                                outap = ps[:, hm * TT + 512: hm * TT + TT]
                                mm(outap, w1_sb[:, 0:2, m * 128:(m + 1) * 128],
                                   cur8[:, :, c0:c0 + 512], True, False, DRM)
                                mm(outap, w1_sb[:, 2:3, m * 128:(m + 1) * 128],
                                   cur8[:, 0:1, c0 + 1:c0 + 513], False, True)
                            for hm in range(2):
                                m = 2 * r + hm
                                c0 = 1 + t0
                                mm(ps[:, hm * TT: hm * TT + 512],
                                   w1_sb[:, 2:3, m * 128:(m + 1) * 128],
                                   cur8[:, 0:1, c0 + 1:c0 + 513], True, False)
                            for hm in range(2):
                                m = 2 * r + hm
                                c0 = 1 + t0
                                mm(ps[:, hm * TT: hm * TT + 512],
                                   w1_sb[:, 0:2, m * 128:(m + 1) * 128],
                                   cur8[:, :, c0:c0 + 512], False, True, DRM)
                        else:
                            for hm in range(2):
                                m = 2 * r + hm
                                for n in range(2):
                                    outap = ps[:, hm * TT + n * 512:
                                               hm * TT + (n + 1) * 512]
                                    c0 = 1 + t0 + n * 512
                                    mm(outap, w1_sb[:, 0:2, m * 128:(m + 1) * 128],
                                       cur8[:, :, c0:c0 + 512], True, False, DRM)
                                    mm(outap, w1_sb[:, 2:3, m * 128:(m + 1) * 128],
                                       cur8[:, 0:1, c0 + 1:c0 + 513], False, True)
                        if not boundary:
                            hh = hpool.tile([128, 2, TT], F8, tag="h2",
                                            name=f"h2b_{it}_{mt}_{r}")
                            nc.scalar.activation(
                                hh[:].rearrange("p two f -> p (two f)"), ps[:],
                                AF.Gelu, scale=1.0 / WS)
                            h2s[r] = hh
                        if r == 0 and pending is not None:
                            pending()
                            pending = None
                        if r in (2, 4, 6):
                            g2_pair(mt, r - 2, h2s[r - 2], h2s[r - 1], accs[mt],
                                    cur, nxt, nxt8, False, False,
                                    f"{it}_{mt}_{r - 2}")
                    # defer pair-67 to the next mt's rule 0, or across the
                    # iteration boundary (split); issue directly on the last one
                    fix = (mt == NMT - 1) and (it < n_evolve - 1)
                    if mt == NMT - 1 and it == n_evolve - 1:
                        g2_pair(mt, 6, h2s[6], h2s[7], accs[mt],
                                cur, nxt, nxt8, False, False, f"{it}_{mt}_6",
                                images=False)
                        # preload the sqrt table the moment the last tanh's
                        # acc is written: the 1.3us load overlaps the final
                        # blend instead of sitting on the LN critical path
                        nc.scalar.sqrt(junk[:], accs[mt][:, 0:1])
                    else:
                        def mk(pair_mt=mt, h2a=h2s[6], h2b=h2s[7], acc=accs[mt],
                               cur_=cur, nxt_=nxt, nxt8_=nxt8, split=fix,
                               fixups=fix, uid=f"{it}_{mt}_6",
                               images=(it < n_evolve - 1)):
                            g2_pair(pair_mt, 6, h2a, h2b, acc, cur_, nxt_, nxt8_,
                                    split, fixups, uid, images=images)
                        pending = mk
                cur, nxt = nxt, cur
                cur8, nxt8 = nxt8, cur8

        # ---- LayerNorm over D + store (reads f16 state `cur`) ----
        # quarter-granular pipeline: per 8-block quarter, transpose+mu-MM (PE)
        # -> Square (ACT) -> x^2 reduce (DVE) -> stats math -> apply
        # (ACT/DVE alternating per quarter) -> f16 DMA, so quarters overlap
        # across all engines and the last quarter's chain is short
        with tc.tile_pool(name="psum2", bufs=4, space="PSUM") as pp2:
            dmaq = (nc.sync, nc.gpsimd, nc.scalar, nc.sync)
            for qt in range(4):
                h0 = qt * 8
                qs = slice(0, 8)
                mu = lnp.tile([128, 8], F32, tag="mu", name=f"mu{qt}")
                v = lnp.tile([128, 8], F32, tag="v", name=f"v{qt}")
                dd = lnp.tile([128, 8], F32, tag="dd", name=f"dd{qt}")
                scr = lnp.tile([128, 8], F32, tag="scr", name=f"scr{qt}")
                rstd = lnp.tile([128, 8], F32, tag="rstd", name=f"rstd{qt}")
                nmr = lnp.tile([128, 8], F32, tag="nmr", name=f"nmr{qt}")
                pst = pp2.tile([128, 1024], F16, tag="lnt", name=f"lnt{qt}")
                mus = pp2.tile([128, 8], F32, tag="mus", name=f"mus{qt}")
                for j in range(8):
                    jj = h0 + j
                    cs = cur[:, 1 + jj * 128: 1 + (jj + 1) * 128]
                    nc.tensor.transpose(
                        pst[:, j * 128:(j + 1) * 128], cs, ident16[:])
                    # per-token sums on the PE: same stationary as the
                    # transpose, one extra N=1 MM vs a DVE reduce
                    mm(mus[:, j:j + 1], cs, ones16[:], True, True)
                # per-block x^2 sums via Square (ACT) + one 3D reduce (DVE)
                sq = lnp.tile([128, 1024], F16, tag="sq", name=f"sq{qt}")
                nc.scalar.activation(sq[:], pst[:], AF.Square)
                s3 = sq[:].rearrange("p (b f) -> p b f", f=128)
                nc.vector.tensor_reduce(v[:, qs].unsqueeze(2), s3,
                                        mybir.AxisListType.X, OP.add)
                nc.vector.tensor_scalar_mul(mu[:, qs], mus[:], 1.0 / D)
                nc.vector.tensor_scalar_mul(dd[:, qs], mu[:, qs], -1.0)
                nc.vector.tensor_mul(dd[:, qs], dd[:, qs], mu[:, qs])
                nc.vector.tensor_scalar(v[:, qs], v[:, qs], 1.0 / D, LN_EPS,
                                        OP.mult, OP.add)
                nc.vector.tensor_add(v[:, qs], v[:, qs], dd[:, qs])
                nc.scalar.sqrt(scr[:, qs], v[:, qs])
                nc.vector.reciprocal(rstd[:, qs], scr[:, qs])
                nc.vector.scalar_tensor_tensor(nmr[:, qs], mu[:, qs], -1.0,
                                               rstd[:, qs], OP.mult, OP.mult)
                # output is f16 (host casts to f32; ~5e-4 rel, negligible);
                # applies alternate ACT/DVE per block so the two engines
                # split each quarter; half-quarter DMAs on sync/gpsimd
                # queues so the final drain overlaps the last applies
                ob = lnp.tile([128, 1024], F16, tag="obig", name=f"obig{qt}")
                for j in range(8):
                    o = ob[:, j * 128:(j + 1) * 128]
                    blk = pst[:, j * 128:(j + 1) * 128]
                    if (qt + j) % 2 == 0 and not apply_gb:
                        nc.scalar.activation(o, blk, AF.Identity,
                                             bias=nmr[:, j:j + 1],
                                             scale=rstd[:, j:j + 1])
                    else:
                        nc.vector.tensor_scalar(o, blk,
                                                rstd[:, j:j + 1],
                                                nmr[:, j:j + 1],
                                                OP.mult, OP.add)
                    if apply_gb:
                        nc.vector.tensor_mul(o, o, gb_sb[:, 0:D])
                        nc.vector.tensor_add(o, o, gb_sb[:, D:2 * D])
                    if j == 3:
                        nc.sync.dma_start(
                            y_d[:, h0:h0 + 4, :],
                            ob[:, 0:512].rearrange("p (j d) -> p j d", j=4))
                nc.gpsimd.dma_start(
                    y_d[:, h0 + 4:h0 + 8, :],
                    ob[:, 512:1024].rearrange("p (j d) -> p j d", j=4))
    nc.compile()
    return nc


def _prep_weights(inputs):
    import ml_dtypes
    W1 = np.asarray(inputs["W1"], np.float32) * WS   # [R, 3D, 2D]
    W2 = np.asarray(inputs["W2"], np.float32) * WS   # [R, 2D, D]
    # w1[kk, k, (2r+hm)*128 + j] = W1[r, k*128 + kk, hm*128 + j]
    w1 = np.ascontiguousarray(
        W1.reshape(R, 3, 128, 2, 128).transpose(2, 1, 0, 3, 4).reshape(128, 3, M1))
    # w2[kk, k, r*128 + d] = W2[r, k*128 + kk, d]
    w2 = np.ascontiguousarray(
        W2.reshape(R, 2, 128, D).transpose(2, 1, 0, 3).reshape(128, 2, R * D))
    return w1.astype(ml_dtypes.float8_e4m3fn), w2.astype(ml_dtypes.float8_e4m3fn)


def kernel(**inputs):
    rule_w, alpha, n_evolve = _selectors(inputs)
    b1 = np.asarray(inputs["b1"], np.float32)
    b2 = np.asarray(inputs["b2"], np.float32)
    assert not b1.any() and not b2.any(), "nonzero rule biases unsupported"
    ln_g = np.asarray(inputs["ln_g"], np.float32)
    ln_b = np.asarray(inputs["ln_b"], np.float32)
    apply_gb = bool((ln_g != 1.0).any() or ln_b.any())

    nc = build_nc(n_evolve, alpha, rule_w, apply_gb=apply_gb)

    import ml_dtypes
    w1, w2 = _prep_weights(inputs)
    x = np.asarray(inputs["cells_state"], np.float32)   # [B, T, D]
    in_maps = []
    for b in range(B):
        xT = np.ascontiguousarray(x[b].T)               # [D, T]
        x16 = xT.astype(np.float16)
        x8 = xT.astype(ml_dtypes.float8_e4m3fn)
        img = np.zeros((128, 2, LDUP), ml_dtypes.float8_e4m3fn)
        img[:, 0, 1:T + 1] = x8
        img[:, 0, T + 1] = x8[:, 0]                     # wrap: x[0]
        img[:, 1, 2:T + 2] = x8
        img[:, 1, 1] = x8[:, T - 1]                     # wrap: x[T-1]
        m = {"x16": x16, "ximg": img, "w1": w1, "w2": w2}
        if apply_gb:
            m["gb"] = np.ascontiguousarray(
                np.stack([np.tile(ln_g, (128, 1)), np.tile(ln_b, (128, 1))]))
        in_maps.append(m)
    res = run_bass_kernel_spmd(nc, in_maps, list(range(B)))
    global LAST_RESULTS
    LAST_RESULTS = res
    # un-permute the partition-major staging layout: y[p, jj, d] -> [T, D]
    out = np.stack([np.asarray(res.results[b]["y"]).transpose(1, 0, 2)
                    .reshape(T, D) for b in range(B)])
    return out.astype(np.float32)

